# revision 20
# baseline (speedup 1.0000x reference)
"""Trainium2 Bass kernel for nn_AttentionBlock (smooth-softmax attention).

  out = smoothsoftmax((x@Wq+bq) @ (y@Wk+bk)^T) @ (y@Wv+bv)
  smoothsoftmax(M) = (0.1*relu(M) + softmax(M)) / rowsum(...)

Strategy (per core, x row-sharded across 8 cores):
  M = XQ @ YK^T has rank 8: M = q8 @ yext^T with q8 = x@Wqk+bqk (host-folded
  Wqk = Wq@[Wk^T|bk], yext = [y|1]).  Per 512-row tile:
    - produce M^T key-chunks [128, R] on PE from q8t (fp16, 8-deep contraction)
    - ACT: eg = exp(M^T) bf16 (pair-sized ops)
    - DVE/Pool alternate: rg = relu(M^T) -> fp8e4
    - PE consumes both against the 8-column yext basis:
        G2^T [8,R] += yext_chunk^T-stationary @ eg      (bf16, 8-col weights)
        G1^T [8,R] += fp8 DoubleRow over chunk pairs @ rg (2x rate)
      row 7 of each = rowsum (ones column) => S and R.
    - tail: O1 = G1 @ (0.1*[Wv;bv]|0.1*e7), O2 = G2 @ ([Wv;bv]|e7) row-major,
      then out = O1[:, :64]/den + O2[:, :64]/(S*den), den = 1 + 0.1*R
  x is transposed + cast to fp16 on the host, so no on-chip transposes at all.
"""

import numpy as np
from contextlib import ExitStack

import concourse.bass as bass
import concourse.mybir as mybir
import concourse.tile as tile

# ----------------------------------------------------------------------------
# Workaround for walrus "Too many sync wait commands" on the TileContext
# kernel-tail Drain: pre-issue the global-clock waits on the sync engine one
# per nop before the drain; the drain itself then needs no waits (SP executes
# in order).
from concourse.vector_clock import ScopedClock, VectorClock


def _drain_and_barrier_split(self, tick_clock, wait_clock):
    gc = tick_clock.global_clock
    n = len(gc)
    procs = [p for p in range(n) if gc[p] > 0]
    for p in procs:
        vec = [gc[q] if q == p else 0 for q in range(n)]
        nop = self.nc.sync.nop(nofuse=True, hint="drain_wait_split")
        wait_clock.add_sem_waits(nop.ins, ScopedClock({None: VectorClock(vec)}))
    self.nc.sync.drain()
    self.nc.all_engine_barrier()
    assert self.sems is not None
    popped = self.nc._tile_sem_poison_stack.pop()
    assert popped is self._sem_poison
    self.nc.clear_and_free_semaphores(list(self.sems.allocated().values()))
    self.nc.all_engine_barrier()


tile.TileContext._drain_and_barrier = _drain_and_barrier_split


def _split_multi_waits(nc, max_waits=1):
    """This walrus build rejects instructions carrying more than one sync
    wait.  Hoist extra waits onto single-wait NoOps on the same engine
    immediately before the instruction (engine streams execute in order,
    so semantics are identical)."""
    for f in nc.m.functions:
        for b in f.blocks:
            out = []
            changed = False
            for inst in b.instructions:
                si = inst.sync_info
                if si is not None and si.on_wait and len(si.on_wait) > max_waits:
                    waits = list(si.on_wait)
                    for w in waits[max_waits:]:
                        out.append(mybir.InstNoOp(
                            name=nc.get_next_instruction_name(),
                            engine=inst.engine,
                            bass_nofuse=True,
                            sync_info=mybir.SyncInfo(on_wait=[w], on_update=[]),
                        ))
                    si.on_wait = waits[:max_waits]
                    changed = True
                out.append(inst)
            if changed:
                b.instructions = out
# ----------------------------------------------------------------------------

F32 = mybir.dt.float32
BF16 = mybir.dt.bfloat16
FP16 = mybir.dt.float16
FP8 = mybir.dt.float8e4

N_CORES = 8
N_FULL = 50000
S_IN = 256
NY = 4096
YDIM = 7
D = 64

ROWS_PER_CORE = (N_FULL + N_CORES - 1) // N_CORES  # 6250
PAD_ROWS = ((ROWS_PER_CORE + 127) // 128) * 128    # 6272

AF = mybir.ActivationFunctionType
ALU = mybir.AluOpType
PM = mybir.MatmulPerfMode


def build_nc(pad_rows=PAD_ROWS, ny=NY, big_tile=512, split_waits=True,
             m_f32r=False):
    """Build the per-core Bass program. All 8 cores run the same program on
    different x shards (y and the projection weights are replicated)."""
    del m_f32r  # compat knob from the old harness; unused
    nc = bass.Bass(trn_type="TRN2")

    nchunks = ny // 128
    npairs = nchunks // 2
    assert ny % 256 == 0

    xt_h = nc.dram_tensor("xT", [S_IN, pad_rows], FP16, kind="ExternalInput")
    ye8t_h = nc.dram_tensor("yext8T", [8, ny], FP16, kind="ExternalInput")
    yec_h = nc.dram_tensor("yextc", [128, nchunks, 8], BF16, kind="ExternalInput")
    yef8_h = nc.dram_tensor("yextf8", [128, npairs, 2, 16], FP8, kind="ExternalInput")
    wqk_h = nc.dram_tensor("wqk", [128, 2, 8], FP16, kind="ExternalInput")
    bqk_h = nc.dram_tensor("bqk", [8, 1], F32, kind="ExternalInput")
    wvb1_h = nc.dram_tensor("wvb1", [8, D + 1], FP16, kind="ExternalInput")
    wvb2_h = nc.dram_tensor("wvb2", [8, D + 1], BF16, kind="ExternalInput")
    out_h = nc.dram_tensor("out", [pad_rows, D], F32, kind="ExternalOutput")

    # row tiles: big_tile-row tiles then a 128-multiple remainder tile
    tiles = []
    r0 = 0
    while r0 + big_tile <= pad_rows:
        tiles.append((r0, big_tile))
        r0 += big_tile
    if r0 < pad_rows:
        assert (pad_rows - r0) % 128 == 0
        tiles.append((r0, pad_rows - r0))

    with tile.TileContext(nc) as tc, ExitStack() as ctx:
        singles = ctx.enter_context(tc.tile_pool(name="singles", bufs=1))
        psum_m = ctx.enter_context(tc.tile_pool(name="psum_m", bufs=2, space="PSUM"))
        psum_g = ctx.enter_context(tc.tile_pool(name="psum_g", bufs=1, space="PSUM"))
        xt_pool = ctx.enter_context(tc.tile_pool(name="xt", bufs=3))
        q8_pool = ctx.enter_context(tc.tile_pool(name="q8", bufs=2))
        eg_pool = ctx.enter_context(tc.tile_pool(name="eg", bufs=5))
        rg_pool = ctx.enter_context(tc.tile_pool(name="rg", bufs=5))
        gs_pool = ctx.enter_context(tc.tile_pool(name="gs", bufs=2))
        tail_pool = ctx.enter_context(tc.tile_pool(name="tail", bufs=2))

        # ------------------------------------------------------------------
        # Constants (once per core)
        # ------------------------------------------------------------------
        wqk = singles.tile([128, 2, 8], FP16)
        nc.sync.dma_start(out=wqk, in_=wqk_h[:, :, :])
        bqk = singles.tile([8, 1], F32)
        nc.sync.dma_start(out=bqk, in_=bqk_h[:, :])
        ye8t = singles.tile([8, ny], FP16)
        nc.sync.dma_start(out=ye8t, in_=ye8t_h[:, :])
        yec = singles.tile([128, nchunks, 8], BF16)
        nc.sync.dma_start(out=yec, in_=yec_h[:, :, :])
        yef8 = singles.tile([128, npairs, 2, 16], FP8)
        nc.sync.dma_start(out=yef8, in_=yef8_h[:, :, :, :])
        wvb1 = singles.tile([8, D + 1], FP16)
        nc.sync.dma_start(out=wvb1, in_=wvb1_h[:, :])
        wvb2 = singles.tile([8, D + 1], BF16)
        nc.sync.dma_start(out=wvb2, in_=wvb2_h[:, :])

        def emit_head(r0, R):
            """Load one row tile of x^T and project to q8^T [8, R] fp16."""
            xt_t = xt_pool.tile([128, 2, R], FP16, tag="xt")
            nc.sync.dma_start(
                out=xt_t,
                in_=xt_h[:, r0:r0 + R].rearrange("(c p) r -> p c r", p=128),
            )
            q8_slot = psum_m.tile([128, 2, R], F32, tag="m")
            q8_ps = q8_slot[0:8, 0, :]
            nc.tensor.matmul(q8_ps, wqk[:, 0, :], xt_t[:, 0, :],
                             start=True, stop=False)
            nc.tensor.matmul(q8_ps, wqk[:, 1, :], xt_t[:, 1, :],
                             start=False, stop=True)
            q8t = q8_pool.tile([8, R], FP16, tag="q8")
            nc.scalar.add(q8t, q8_ps, bqk)
            return q8t

        next_q8t = emit_head(*tiles[0])

        # ------------------------------------------------------------------
        # Main loop over row tiles.  Two decoupling tricks keep the PE stream
        # gapless (the PE only reaches its 2.4GHz p-state after ~3us without
        # a stall):
        #  - consumption lag: the G matmuls for pair j are emitted after the
        #    production of pair j+LAG, so exp/relu have LAG pair-times of
        #    slack before the PE needs their output
        #  - deferred tails: tile ti's tail (O matmuls + combine + store) is
        #    emitted in the middle of tile ti+1, when its G drains are long
        #    done
        # ------------------------------------------------------------------
        LAG = 3

        def emit_tail(r0, R, g1s, g2s):
            C = R // 128
            o1 = psum_g.tile([128, C, D + 1], F32, tag="o1")
            o2 = psum_g.tile([128, C, D + 1], F32, tag="o2")
            for sc in range(C):
                nc.tensor.matmul(
                    o1[:, sc, :], g1s[:, sc * 128:(sc + 1) * 128], wvb1,
                    start=(sc == 0), stop=(sc == C - 1),
                    skip_group_check=True,
                )
                nc.tensor.matmul(
                    o2[:, sc, :], g2s[:, sc * 128:(sc + 1) * 128], wvb2,
                    start=(sc == 0), stop=(sc == C - 1),
                    skip_group_check=True,
                )

            # drain O psums to SBUF so the Pool engine can do the combine
            o1s = tail_pool.tile([128, C, D + 1], F32, tag="o1s")
            o2s = tail_pool.tile([128, C, D + 1], F32, tag="o2s")
            nc.vector.tensor_copy(out=o1s, in_=o1)
            nc.vector.tensor_copy(out=o2s, in_=o2)

            # den = 1 + 0.1*R  (o1 col 64 is already 0.1*R: wvb1 pre-scaled)
            den = tail_pool.tile([128, C], F32, tag="den")
            rec = tail_pool.tile([128, C], F32, tag="rec")
            sd = tail_pool.tile([128, C], F32, tag="sd")
            bb = tail_pool.tile([128, C], F32, tag="bb")
            ot = tail_pool.tile([128, C, D], F32, tag="ot")
            t1 = tail_pool.tile([128, D], F32, tag="t1")
            t2 = tail_pool.tile([128, D], F32, tag="t2")
            nc.vector.tensor_scalar_add(out=den, in0=o1s[:, :, D], scalar1=1.0)
            nc.vector.reciprocal(out=rec, in_=den)
            nc.vector.tensor_mul(out=sd, in0=o2s[:, :, D], in1=den)
            nc.vector.reciprocal(out=bb, in_=sd)
            for sc in range(C):
                nc.gpsimd.tensor_scalar_mul(
                    out=t1, in0=o1s[:, sc, 0:D], scalar1=rec[:, sc:sc + 1])
                nc.gpsimd.tensor_scalar_mul(
                    out=t2, in0=o2s[:, sc, 0:D], scalar1=bb[:, sc:sc + 1])
                nc.gpsimd.tensor_add(out=ot[:, sc, :], in0=t1, in1=t2)

            nc.sync.dma_start(
                out=out_h[r0:r0 + R, :].rearrange("(s p) d -> p s d", p=128),
                in_=ot,
            )

        # Flat loop over (tile, pair) so the consumption lag carries across
        # tile boundaries -- the PE stream has no per-tile flush bubble, which
        # would re-throttle the HAM clock gate.
        all_pairs = [(ti, pj) for ti in range(len(tiles)) for pj in range(npairs)]
        ctx_by_tile = {}
        pending_tail = None
        lagq = []

        def emit_lagged(lagged, c):
            (lti, lpj, leg, lrg) = lagged
            lg1t, lg2t = ctx_by_tile[lti]["g"]
            if c < 2:
                lj = 2 * lpj + c
                nc.tensor.matmul(
                    lg2t, yec[:, lj, :], leg[:, c, :],
                    start=(lj == 0), stop=(lj == nchunks - 1),
                    skip_group_check=True,
                )
            else:
                nc.tensor.matmul(
                    lg1t, yef8[:, lpj, :, :], lrg,
                    start=(lpj == 0), stop=(lpj == npairs - 1),
                    perf_mode=PM.DoubleRow, skip_group_check=True,
                )
                if lpj == npairs - 1:
                    # this tile's accumulation is complete: drain it
                    R_l = tiles[lti][1]
                    g1s = gs_pool.tile([8, R_l], FP16, tag="g1s")
                    g2s = gs_pool.tile([8, R_l], BF16, tag="g2s")
                    nc.vector.tensor_copy(out=g1s, in_=lg1t[0:8, :])
                    nc.vector.tensor_copy(out=g2s, in_=lg2t)
                    ctx_by_tile[lti]["tail"] = (tiles[lti][0], R_l, g1s, g2s)

        for ti, pj in all_pairs:
            r0, R = tiles[ti]
            if pj == 0:
                q8t = next_q8t
                g1t = psum_g.tile([16, R], F32, tag="g1")
                g2t = psum_g.tile([8, R], F32, tag="g2")
                ctx_by_tile[ti] = {"g": (g1t, g2t), "q8": q8t, "tail": None}
            q8t = ctx_by_tile[ti]["q8"]
            g1t, g2t = ctx_by_tile[ti]["g"]

            lagged = lagq.pop(0) if len(lagq) >= LAG else None
            half_is_bank = (R * 4) >= 2048
            mt = psum_m.tile([128, 2, R], F32, tag="m")
            for c in range(2):
                j = 2 * pj + c
                nc.tensor.matmul(
                    mt[:, c, :], ye8t[:, j * 128:(j + 1) * 128], q8t,
                    start=(c == 0 or half_is_bank), stop=True,
                    skip_group_check=True,
                )
                if lagged is not None:
                    emit_lagged(lagged, c)
            if lagged is not None:
                emit_lagged(lagged, 2)
            eg = eg_pool.tile([128, 2, R], BF16, tag="eg")
            nc.scalar.activation(out=eg.rearrange("p a b -> p (a b)"),
                                 in_=mt.rearrange("p a b -> p (a b)"),
                                 func=AF.Exp)
            rg = rg_pool.tile([128, 2, R], FP8, tag="rg")
            if pj in (5, 11):
                # ACT helps with relu: DVE alone can't keep up with 16
                # relu pairs + drains per tile
                nc.scalar.activation(out=rg.rearrange("p a b -> p (a b)"),
                                     in_=mt.rearrange("p a b -> p (a b)"),
                                     func=AF.Relu)
            else:
                nc.vector.tensor_scalar_max(
                    out=rg.rearrange("p a b -> p (a b)"),
                    in0=mt.rearrange("p a b -> p (a b)"), scalar1=0.0)
            lagq.append((ti, pj, eg, rg))

            if pj == 4 and ti + 1 < len(tiles):
                # software pipeline: emit next tile's head mid-loop
                next_q8t = emit_head(*tiles[ti + 1])
            if pj == 7 and ti > 0 and ctx_by_tile[ti - 1]["tail"] is not None:
                emit_tail(*ctx_by_tile[ti - 1]["tail"])
                ctx_by_tile[ti - 1]["tail"] = None

        while lagq:
            lagged = lagq.pop(0)
            for c in range(3):
                emit_lagged(lagged, c)
        for ti in range(len(tiles)):
            t = ctx_by_tile[ti]["tail"]
            if t is not None:
                emit_tail(*t)

    if split_waits:
        _split_multi_waits(nc)
    return nc


_NC_CACHE = {}

# test-harness knobs (the grading harness uses the defaults)
TRACE = False
LAST_RESULT = None


def _get_nc(pad_rows, ny):
    key = (pad_rows, ny)
    if key not in _NC_CACHE:
        _NC_CACHE[key] = build_nc(pad_rows, ny)
    return _NC_CACHE[key]


def kernel(x, y, Wq, bq, Wk, bk, Wv, bv):
    from concourse.bass_utils import run_bass_kernel_spmd

    x = np.ascontiguousarray(np.asarray(x, dtype=np.float32))
    y = np.ascontiguousarray(np.asarray(y, dtype=np.float32))
    Wq = np.asarray(Wq, np.float32)
    bq = np.asarray(bq, np.float32)
    Wk = np.asarray(Wk, np.float32)
    bk = np.asarray(bk, np.float32)
    Wv = np.asarray(Wv, np.float32)
    bv = np.asarray(bv, np.float32)

    n = x.shape[0]
    rows_per_core = (n + N_CORES - 1) // N_CORES
    pad_rows = ((rows_per_core + 127) // 128) * 128
    ny = y.shape[0]
    nchunks = ny // 128
    npairs = nchunks // 2

    nc = _get_nc(pad_rows, ny)

    import ml_dtypes
    f8np = mybir.dt.np(FP8)
    bf16np = mybir.dt.np(BF16)

    # host-side folding of the rank-8 score structure
    Wkb = np.concatenate([Wk.T, bk[:, None]], axis=1)          # [64, 8]
    Wqk = (Wq.astype(np.float64) @ Wkb.astype(np.float64))     # [256, 8]
    bqk = (bq.astype(np.float64) @ Wkb.astype(np.float64))     # [8]
    yext = np.concatenate([y, np.ones((ny, 1), np.float32)], axis=1)  # [ny, 8]
    wvb = np.concatenate([Wv, bv[None, :]], axis=0)            # [8, 64]
    e7 = np.zeros((8, 1), np.float32)
    e7[7, 0] = 1.0
    wvb1 = 0.1 * np.concatenate([wvb, e7], axis=1)             # [8, 65]
    wvb2 = np.concatenate([wvb, e7], axis=1)                   # [8, 65]

    common = {
        "yext8T": np.ascontiguousarray(yext.T).astype(np.float16),
        "yextc": np.ascontiguousarray(
            yext.reshape(nchunks, 128, 8).transpose(1, 0, 2)).astype(bf16np),
        "yextf8": np.ascontiguousarray(
            np.concatenate([yext.reshape(npairs, 2, 128, 8),
                            np.zeros((npairs, 2, 128, 8), np.float32)],
                           axis=3).transpose(2, 0, 1, 3)).astype(f8np),
        "wqk": np.ascontiguousarray(
            Wqk.astype(np.float32).reshape(2, 128, 8).transpose(1, 0, 2)
        ).astype(np.float16),
        "bqk": np.ascontiguousarray(bqk.astype(np.float32)[:, None]),
        "wvb1": wvb1.astype(np.float16),
        "wvb2": wvb2.astype(bf16np),
    }

    xp = np.zeros((N_CORES, S_IN, pad_rows), np.float16)
    for c in range(N_CORES):
        lo = c * rows_per_core
        hi = min(lo + rows_per_core, n)
        xp[c, :, 0:hi - lo] = x[lo:hi].T

    in_maps = [{"xT": xp[c], **common} for c in range(N_CORES)]
    kwargs = {}
    if TRACE:
        import shutil
        shutil.rmtree("/tmp/kern_trace", ignore_errors=True)
        kwargs = dict(trace=True, tmpdir="/tmp/kern_trace")
    res = run_bass_kernel_spmd(nc, in_maps, core_ids=list(range(N_CORES)), **kwargs)
    global LAST_RESULT
    LAST_RESULT = res

    out = np.empty((n, D), np.float32)
    for c in range(N_CORES):
        lo = c * rows_per_core
        hi = min(lo + rows_per_core, n)
        out[lo:hi] = res.results[c]["out"][0:hi - lo]
    return out


# revision 21
# speedup vs baseline: 1.2315x; 1.2315x over previous
"""Trainium2 Bass kernel for nn_AttentionBlock (smooth-softmax attention).

  out = smoothsoftmax((x@Wq+bq) @ (y@Wk+bk)^T) @ (y@Wv+bv)
  smoothsoftmax(M) = (0.1*relu(M) + softmax(M)) / rowsum(...)

Strategy (per core, x row-sharded across 8 cores):
  M = XQ @ YK^T has rank 8: M = q8 @ yext^T with q8 = x@Wqk+bqk (host-folded
  Wqk = Wq@[Wk^T|bk], yext = [y|1]).  Per 512-row tile:
    - produce M^T key-chunks [128, R] on PE from q8t (fp16, 8-deep contraction)
    - ACT: eg = exp(M^T) bf16 (pair-sized ops)
    - DVE/Pool alternate: rg = relu(M^T) -> fp8e4
    - PE consumes both against the 8-column yext basis:
        G2^T [8,R] += yext_chunk^T-stationary @ eg      (bf16, 8-col weights)
        G1^T [8,R] += fp8 DoubleRow over chunk pairs @ rg (2x rate)
      row 7 of each = rowsum (ones column) => S and R.
    - tail: O1 = G1 @ (0.1*[Wv;bv]|0.1*e7), O2 = G2 @ ([Wv;bv]|e7) row-major,
      then out = O1[:, :64]/den + O2[:, :64]/(S*den), den = 1 + 0.1*R
  x is transposed + cast to fp16 on the host, so no on-chip transposes at all.
"""

import numpy as np
from contextlib import ExitStack

import concourse.bass as bass
import concourse.mybir as mybir
import concourse.tile as tile

# ----------------------------------------------------------------------------
# Workaround for walrus "Too many sync wait commands" on the TileContext
# kernel-tail Drain: pre-issue the global-clock waits on the sync engine one
# per nop before the drain; the drain itself then needs no waits (SP executes
# in order).
from concourse.vector_clock import ScopedClock, VectorClock


def _drain_and_barrier_split(self, tick_clock, wait_clock):
    gc = tick_clock.global_clock
    n = len(gc)
    procs = [p for p in range(n) if gc[p] > 0]
    for p in procs:
        vec = [gc[q] if q == p else 0 for q in range(n)]
        nop = self.nc.sync.nop(nofuse=True, hint="drain_wait_split")
        wait_clock.add_sem_waits(nop.ins, ScopedClock({None: VectorClock(vec)}))
    self.nc.sync.drain()
    self.nc.all_engine_barrier()
    assert self.sems is not None
    popped = self.nc._tile_sem_poison_stack.pop()
    assert popped is self._sem_poison
    self.nc.clear_and_free_semaphores(list(self.sems.allocated().values()))
    self.nc.all_engine_barrier()


tile.TileContext._drain_and_barrier = _drain_and_barrier_split


def _split_multi_waits(nc, max_waits=1):
    """This walrus build rejects instructions carrying more than one sync
    wait.  Hoist extra waits onto single-wait NoOps on the same engine
    immediately before the instruction (engine streams execute in order,
    so semantics are identical)."""
    for f in nc.m.functions:
        for b in f.blocks:
            out = []
            changed = False
            for inst in b.instructions:
                si = inst.sync_info
                if si is not None and si.on_wait and len(si.on_wait) > max_waits:
                    waits = list(si.on_wait)
                    for w in waits[max_waits:]:
                        out.append(mybir.InstNoOp(
                            name=nc.get_next_instruction_name(),
                            engine=inst.engine,
                            bass_nofuse=True,
                            sync_info=mybir.SyncInfo(on_wait=[w], on_update=[]),
                        ))
                    si.on_wait = waits[:max_waits]
                    changed = True
                out.append(inst)
            if changed:
                b.instructions = out
# ----------------------------------------------------------------------------

F32 = mybir.dt.float32
BF16 = mybir.dt.bfloat16
FP16 = mybir.dt.float16
FP8 = mybir.dt.float8e4

N_CORES = 8
N_FULL = 50000
S_IN = 256
NY = 4096
YDIM = 7
D = 64

ROWS_PER_CORE = (N_FULL + N_CORES - 1) // N_CORES  # 6250
PAD_ROWS = ((ROWS_PER_CORE + 127) // 128) * 128    # 6272

AF = mybir.ActivationFunctionType
ALU = mybir.AluOpType
PM = mybir.MatmulPerfMode


def build_nc(pad_rows=PAD_ROWS, ny=NY, big_tile=512, split_waits=True,
             m_f32r=False):
    """Build the per-core Bass program. All 8 cores run the same program on
    different x shards (y and the projection weights are replicated)."""
    del m_f32r  # compat knob from the old harness; unused
    nc = bass.Bass(trn_type="TRN2")

    nchunks = ny // 128
    npairs = nchunks // 2
    assert ny % 256 == 0

    xt_h = nc.dram_tensor("xT", [S_IN, pad_rows], FP16, kind="ExternalInput")
    ye8t_h = nc.dram_tensor("yext8T", [8, ny], FP16, kind="ExternalInput")
    yec_h = nc.dram_tensor("yextc", [128, nchunks, 8], BF16, kind="ExternalInput")
    yef8_h = nc.dram_tensor("yextf8", [128, npairs, 2, 16], FP8, kind="ExternalInput")
    wqk_h = nc.dram_tensor("wqk", [128, 2, 8], FP16, kind="ExternalInput")
    bqk_h = nc.dram_tensor("bqk", [8, 1], F32, kind="ExternalInput")
    wvb1_h = nc.dram_tensor("wvb1", [8, D + 1], FP16, kind="ExternalInput")
    wvb2_h = nc.dram_tensor("wvb2", [8, D + 1], BF16, kind="ExternalInput")
    out_h = nc.dram_tensor("out", [pad_rows, D], F32, kind="ExternalOutput")

    # row tiles: big_tile-row tiles then a 128-multiple remainder tile
    tiles = []
    r0 = 0
    while r0 + big_tile <= pad_rows:
        tiles.append((r0, big_tile))
        r0 += big_tile
    if r0 < pad_rows:
        assert (pad_rows - r0) % 128 == 0
        tiles.append((r0, pad_rows - r0))

    with tile.TileContext(nc) as tc, ExitStack() as ctx:
        singles = ctx.enter_context(tc.tile_pool(name="singles", bufs=1))
        psum_m = ctx.enter_context(tc.tile_pool(name="psum_m", bufs=3, space="PSUM"))
        psum_g = ctx.enter_context(tc.tile_pool(name="psum_g", bufs=1, space="PSUM"))
        xt_pool = ctx.enter_context(tc.tile_pool(name="xt", bufs=3))
        q8_pool = ctx.enter_context(tc.tile_pool(name="q8", bufs=2))
        eg_pool = ctx.enter_context(tc.tile_pool(name="eg", bufs=6))
        rg_pool = ctx.enter_context(tc.tile_pool(name="rg", bufs=6))
        gs_pool = ctx.enter_context(tc.tile_pool(name="gs", bufs=2))
        tail_pool = ctx.enter_context(tc.tile_pool(name="tail", bufs=2))

        # ------------------------------------------------------------------
        # Constants (once per core)
        # ------------------------------------------------------------------
        wqk = singles.tile([128, 2, 8], FP16)
        nc.sync.dma_start(out=wqk, in_=wqk_h[:, :, :])
        bqk = singles.tile([8, 1], F32)
        nc.sync.dma_start(out=bqk, in_=bqk_h[:, :])
        ye8t = singles.tile([8, ny], FP16)
        nc.sync.dma_start(out=ye8t, in_=ye8t_h[:, :])
        yec = singles.tile([128, nchunks, 8], BF16)
        nc.sync.dma_start(out=yec, in_=yec_h[:, :, :])
        yef8 = singles.tile([128, npairs, 2, 16], FP8)
        nc.sync.dma_start(out=yef8, in_=yef8_h[:, :, :, :])
        wvb1 = singles.tile([8, D + 1], FP16)
        nc.sync.dma_start(out=wvb1, in_=wvb1_h[:, :])
        wvb2 = singles.tile([8, D + 1], BF16)
        nc.sync.dma_start(out=wvb2, in_=wvb2_h[:, :])

        def emit_head(r0, R):
            """Load one row tile of x^T and project to q8^T [8, R] fp16."""
            xt_t = xt_pool.tile([128, 2, R], FP16, tag="xt")
            nc.sync.dma_start(
                out=xt_t,
                in_=xt_h[:, r0:r0 + R].rearrange("(c p) r -> p c r", p=128),
            )
            q8_slot = psum_m.tile([128, 2, R], F32, tag="m")
            q8_ps = q8_slot[0:8, 0, :]
            nc.tensor.matmul(q8_ps, wqk[:, 0, :], xt_t[:, 0, :],
                             start=True, stop=False)
            nc.tensor.matmul(q8_ps, wqk[:, 1, :], xt_t[:, 1, :],
                             start=False, stop=True)
            q8t = q8_pool.tile([8, R], FP16, tag="q8")
            nc.scalar.add(q8t, q8_ps, bqk)
            return q8t

        next_q8t = emit_head(*tiles[0])

        # ------------------------------------------------------------------
        # Main loop over row tiles.  Two decoupling tricks keep the PE stream
        # gapless (the PE only reaches its 2.4GHz p-state after ~3us without
        # a stall):
        #  - consumption lag: the G matmuls for pair j are emitted after the
        #    production of pair j+LAG, so exp/relu have LAG pair-times of
        #    slack before the PE needs their output
        #  - deferred tails: tile ti's tail (O matmuls + combine + store) is
        #    emitted in the middle of tile ti+1, when its G drains are long
        #    done
        # ------------------------------------------------------------------
        LAG = 4

        def emit_tail(r0, R, g1s, g2s):
            C = R // 128
            # O1/O2 live in one mt-ring slot: O1 fills bank 0 ([:, 0, :]),
            # O2 bank 1, so each bank is zeroed by exactly one start=True.
            o12 = psum_m.tile([128, 2, R], F32, tag="m")
            one_bank = (R * 4) < 2048
            for sc in range(C):
                nc.tensor.matmul(
                    o12[:, 0, sc * 128:sc * 128 + D + 1],
                    g1s[:, sc * 128:(sc + 1) * 128], wvb1,
                    start=(sc == 0), stop=(sc == C - 1),
                    skip_group_check=True,
                )
                nc.tensor.matmul(
                    o12[:, 1, sc * 128:sc * 128 + D + 1],
                    g2s[:, sc * 128:(sc + 1) * 128], wvb2,
                    start=(sc == 0 and not one_bank), stop=(sc == C - 1),
                    skip_group_check=True,
                )

            # drain O psums to SBUF so the Pool engine can do the combine
            o1s = tail_pool.tile([128, C, D + 1], F32, tag="o1s")
            o2s = tail_pool.tile([128, C, D + 1], F32, tag="o2s")
            nc.vector.tensor_copy(
                out=o1s, in_=o12[:, 0, :].rearrange("p (c x) -> p c x", x=128)[:, :, 0:D + 1])
            nc.vector.tensor_copy(
                out=o2s, in_=o12[:, 1, :].rearrange("p (c x) -> p c x", x=128)[:, :, 0:D + 1])

            # den = 1 + 0.1*R  (o1 col 64 is already 0.1*R: wvb1 pre-scaled)
            den = tail_pool.tile([128, C], F32, tag="den")
            rec = tail_pool.tile([128, C], F32, tag="rec")
            sd = tail_pool.tile([128, C], F32, tag="sd")
            bb = tail_pool.tile([128, C], F32, tag="bb")
            ot = tail_pool.tile([128, C, D], F32, tag="ot")
            t1 = tail_pool.tile([128, D], F32, tag="t1")
            t2 = tail_pool.tile([128, D], F32, tag="t2")
            nc.vector.tensor_scalar_add(out=den, in0=o1s[:, :, D], scalar1=1.0)
            nc.vector.reciprocal(out=rec, in_=den)
            nc.vector.tensor_mul(out=sd, in0=o2s[:, :, D], in1=den)
            nc.vector.reciprocal(out=bb, in_=sd)
            for sc in range(C):
                nc.gpsimd.tensor_scalar_mul(
                    out=t1, in0=o1s[:, sc, 0:D], scalar1=rec[:, sc:sc + 1])
                nc.gpsimd.tensor_scalar_mul(
                    out=t2, in0=o2s[:, sc, 0:D], scalar1=bb[:, sc:sc + 1])
                nc.gpsimd.tensor_add(out=ot[:, sc, :], in0=t1, in1=t2)

            nc.sync.dma_start(
                out=out_h[r0:r0 + R, :].rearrange("(s p) d -> p s d", p=128),
                in_=ot,
            )

        # Flat loop over (tile, pair) so the consumption lag carries across
        # tile boundaries -- the PE stream has no per-tile flush bubble, which
        # would re-throttle the HAM clock gate.
        all_pairs = [(ti, pj) for ti in range(len(tiles)) for pj in range(npairs)]
        ctx_by_tile = {}
        pending_tail = None
        lagq = []

        def emit_lagged(lagged, c):
            (lti, lpj, leg, lrg) = lagged
            lg1t, lg2t = ctx_by_tile[lti]["g"]
            if c < 2:
                lj = 2 * lpj + c
                nc.tensor.matmul(
                    lg2t, yec[:, lj, :], leg[:, c, :],
                    start=(lj == 0), stop=(lj == nchunks - 1),
                    skip_group_check=True,
                )
            else:
                nc.tensor.matmul(
                    lg1t, yef8[:, lpj, :, :], lrg,
                    start=(lpj == 0), stop=(lpj == npairs - 1),
                    perf_mode=PM.DoubleRow, skip_group_check=True,
                )
                if lpj == npairs - 1:
                    # this tile's accumulation is complete: drain it
                    R_l = tiles[lti][1]
                    g1s = gs_pool.tile([8, R_l], FP16, tag="g1s")
                    g2s = gs_pool.tile([8, R_l], BF16, tag="g2s")
                    nc.vector.tensor_copy(out=g1s, in_=lg1t[0:8, :])
                    nc.vector.tensor_copy(out=g2s, in_=lg2t)
                    ctx_by_tile[lti]["tail"] = (tiles[lti][0], R_l, g1s, g2s)

        for ti, pj in all_pairs:
            r0, R = tiles[ti]
            if pj == 0:
                q8t = next_q8t
                g1t = psum_g.tile([16, R], F32, tag="g1")
                g2t = psum_g.tile([8, R], F32, tag="g2")
                ctx_by_tile[ti] = {"g": (g1t, g2t), "q8": q8t, "tail": None}
            q8t = ctx_by_tile[ti]["q8"]
            g1t, g2t = ctx_by_tile[ti]["g"]

            lagged = lagq.pop(0) if len(lagq) >= LAG else None
            half_is_bank = (R * 4) >= 2048
            mt = psum_m.tile([128, 2, R], F32, tag="m")
            for c in range(2):
                j = 2 * pj + c
                nc.tensor.matmul(
                    mt[:, c, :], ye8t[:, j * 128:(j + 1) * 128], q8t,
                    start=(c == 0 or half_is_bank), stop=True,
                    skip_group_check=True,
                )
                if lagged is not None:
                    emit_lagged(lagged, c)
            if lagged is not None:
                emit_lagged(lagged, 2)
            eg = eg_pool.tile([128, 2, R], BF16, tag="eg")
            nc.scalar.activation(out=eg.rearrange("p a b -> p (a b)"),
                                 in_=mt.rearrange("p a b -> p (a b)"),
                                 func=AF.Exp)
            rg = rg_pool.tile([128, 2, R], FP8, tag="rg")
            if pj in (5, 11):
                # ACT helps with relu: DVE alone can't keep up with 16
                # relu pairs + drains per tile
                nc.scalar.activation(out=rg.rearrange("p a b -> p (a b)"),
                                     in_=mt.rearrange("p a b -> p (a b)"),
                                     func=AF.Relu)
            else:
                nc.vector.tensor_scalar_max(
                    out=rg.rearrange("p a b -> p (a b)"),
                    in0=mt.rearrange("p a b -> p (a b)"), scalar1=0.0)
            lagq.append((ti, pj, eg, rg))

            if pj == 4 and ti + 1 < len(tiles):
                # software pipeline: emit next tile's head mid-loop
                next_q8t = emit_head(*tiles[ti + 1])
            if pj == 7 and ti > 0 and ctx_by_tile[ti - 1]["tail"] is not None:
                emit_tail(*ctx_by_tile[ti - 1]["tail"])
                ctx_by_tile[ti - 1]["tail"] = None

        while lagq:
            lagged = lagq.pop(0)
            for c in range(3):
                emit_lagged(lagged, c)
        for ti in range(len(tiles)):
            t = ctx_by_tile[ti]["tail"]
            if t is not None:
                emit_tail(*t)

    if split_waits:
        _split_multi_waits(nc)
    return nc


_NC_CACHE = {}

# test-harness knobs (the grading harness uses the defaults)
TRACE = False
LAST_RESULT = None


def _get_nc(pad_rows, ny):
    key = (pad_rows, ny)
    if key not in _NC_CACHE:
        _NC_CACHE[key] = build_nc(pad_rows, ny)
    return _NC_CACHE[key]


def kernel(x, y, Wq, bq, Wk, bk, Wv, bv):
    from concourse.bass_utils import run_bass_kernel_spmd

    x = np.ascontiguousarray(np.asarray(x, dtype=np.float32))
    y = np.ascontiguousarray(np.asarray(y, dtype=np.float32))
    Wq = np.asarray(Wq, np.float32)
    bq = np.asarray(bq, np.float32)
    Wk = np.asarray(Wk, np.float32)
    bk = np.asarray(bk, np.float32)
    Wv = np.asarray(Wv, np.float32)
    bv = np.asarray(bv, np.float32)

    n = x.shape[0]
    rows_per_core = (n + N_CORES - 1) // N_CORES
    pad_rows = ((rows_per_core + 127) // 128) * 128
    ny = y.shape[0]
    nchunks = ny // 128
    npairs = nchunks // 2

    nc = _get_nc(pad_rows, ny)

    import ml_dtypes
    f8np = mybir.dt.np(FP8)
    bf16np = mybir.dt.np(BF16)

    # host-side folding of the rank-8 score structure
    Wkb = np.concatenate([Wk.T, bk[:, None]], axis=1)          # [64, 8]
    Wqk = (Wq.astype(np.float64) @ Wkb.astype(np.float64))     # [256, 8]
    bqk = (bq.astype(np.float64) @ Wkb.astype(np.float64))     # [8]
    yext = np.concatenate([y, np.ones((ny, 1), np.float32)], axis=1)  # [ny, 8]
    wvb = np.concatenate([Wv, bv[None, :]], axis=0)            # [8, 64]
    e7 = np.zeros((8, 1), np.float32)
    e7[7, 0] = 1.0
    wvb1 = 0.1 * np.concatenate([wvb, e7], axis=1)             # [8, 65]
    wvb2 = np.concatenate([wvb, e7], axis=1)                   # [8, 65]

    common = {
        "yext8T": np.ascontiguousarray(yext.T).astype(np.float16),
        "yextc": np.ascontiguousarray(
            yext.reshape(nchunks, 128, 8).transpose(1, 0, 2)).astype(bf16np),
        "yextf8": np.ascontiguousarray(
            np.concatenate([yext.reshape(npairs, 2, 128, 8),
                            np.zeros((npairs, 2, 128, 8), np.float32)],
                           axis=3).transpose(2, 0, 1, 3)).astype(f8np),
        "wqk": np.ascontiguousarray(
            Wqk.astype(np.float32).reshape(2, 128, 8).transpose(1, 0, 2)
        ).astype(np.float16),
        "bqk": np.ascontiguousarray(bqk.astype(np.float32)[:, None]),
        "wvb1": wvb1.astype(np.float16),
        "wvb2": wvb2.astype(bf16np),
    }

    xp = np.zeros((N_CORES, S_IN, pad_rows), np.float16)
    for c in range(N_CORES):
        lo = c * rows_per_core
        hi = min(lo + rows_per_core, n)
        xp[c, :, 0:hi - lo] = x[lo:hi].T

    in_maps = [{"xT": xp[c], **common} for c in range(N_CORES)]
    kwargs = {}
    if TRACE:
        import shutil
        shutil.rmtree("/tmp/kern_trace", ignore_errors=True)
        kwargs = dict(trace=True, tmpdir="/tmp/kern_trace")
    res = run_bass_kernel_spmd(nc, in_maps, core_ids=list(range(N_CORES)), **kwargs)
    global LAST_RESULT
    LAST_RESULT = res

    out = np.empty((n, D), np.float32)
    for c in range(N_CORES):
        lo = c * rows_per_core
        hi = min(lo + rows_per_core, n)
        out[lo:hi] = res.results[c]["out"][0:hi - lo]
    return out
